# revision 1
# baseline (speedup 1.0000x reference)
"""Multi-head attention (b=2, s=2048, h=1024, 16 heads x 64) on 8 NeuronCores.

Sharding: tensor-parallel over heads. Core c owns heads {2c, 2c+1}:
  - qkv projection columns c*128:(c+1)*128 of each of Q/K/V blocks
  - w_out rows c*128:(c+1)*128
Each core computes a full [4096, 1024] partial of the output projection;
the host sums the 8 partials and adds the bias corrections.

Algebraic simplifications (exact up to float rounding):
  - k bias dropped: adds a per-query constant to logits -> softmax invariant.
  - v bias dropped in-kernel: contributes bv @ w_out (a constant row) to the
    output; added on the host together with b_out.
  - 1/sqrt(64) folded into wq/bq on the host.
  - softmax without max subtraction (|logits| <= ~2.1 for this distribution).

Per-core kernel (S^T scheme, feature-on-partition):
  xt = x^T in SBUF [128, 8, 4096] bf16 (hidden on partitions)
  Q^T, K^T per batch [128, 2048] bf16, then per-head row-duplicated so
  consecutive-kt S^T matmuls can row-tile onto disjoint PE row groups.
  V natural per batch [token, vcol] bf16 with a ones column per head.
  S^T tile [k 128, q 512] = K^T_h-slice x Q^T_h   (K=64, kt pairs row-tiled)
  P^T = exp(S^T) on ScalarE per kt-group of 2 (one ACTIVATE per [128, 1024])
  O^T_aug [65, 512] += V_aug-slices x P^T        (row 64 = softmax sums)
  epilogue per 128-q tile: PE-transpose O^T -> O, scale rows by 1/sum,
  transpose back, out [q 128, 512] = O_scaled^T x w_out, DVE evac, DMA out.

Scheduling: engines execute a static per-engine order, so the emission order
IS the schedule. The attention backbone is software-pipelined (S-pair and exp
of group g2, then AV of group g2-1, so the PE never in-order-stalls waiting
for exp), and a filler queue injects stage-A units of the other batch and
deferred epilogue units into the backbone's exp-wait bubbles.
"""

import contextlib
import sys
from collections import deque

import numpy as np

sys.path.insert(0, "/opt/trn_rl_repo")

import ml_dtypes  # noqa: E402

import concourse.bass as bass  # noqa: E402
import concourse.tile as tile  # noqa: E402
from concourse import bacc, mybir  # noqa: E402
from concourse.bass_utils import run_bass_kernel_spmd  # noqa: E402
from concourse.masks import make_identity  # noqa: E402

BF16 = mybir.dt.bfloat16
F32 = mybir.dt.float32
AF = mybir.ActivationFunctionType

B = 2
S = 2048
T = B * S          # 4096 tokens
H = 1024           # hidden
HD = 64            # head dim
N_CORES = 8

_program_cache = {}


class Ctx:
    pass


class Filler:
    """FIFO of generators; pull() advances the head generator one unit."""

    def __init__(self):
        self.q = deque()

    def add(self, gen):
        self.q.append(gen)

    def add_front(self, gen):
        self.q.appendleft(gen)

    def pull(self, n=1):
        while n > 0 and self.q:
            try:
                next(self.q[0])
                n -= 1
            except StopIteration:
                self.q.popleft()

    def drain(self):
        while self.q:
            self.pull()


def qk_units(nc, c, b):
    """Q^T/K^T projection for batch b in 512-token groups + head duplication."""
    for gl in range(4):
        g = b * 4 + gl
        sl = slice(g * 512, (g + 1) * 512)       # global token slice (for xt)
        ll = slice(gl * 512, (gl + 1) * 512)     # local token slice (per-batch)
        psq = c.psA.tile([128, 512], F32, tag="mm", name=f"psq{g}")
        for o in range(8):
            nc.tensor.matmul(
                psq[:], c.wq_sb[:, o, :], c.xt_sb[:, o, sl],
                start=(o == 0), stop=(o == 7),
            )
        nc.scalar.activation(c.QTs[b][:, ll], psq[:], AF.Identity, bias=c.bq_sb[:])
        for h in range(2):
            src = slice(h * 64, (h + 1) * 64)
            nc.vector.tensor_copy(c.QTd[b][h][0:64, ll], c.QTs[b][src, ll])
            nc.vector.tensor_copy(c.QTd[b][h][64:128, ll], c.QTs[b][src, ll])
        yield
        psk = c.psA.tile([128, 512], F32, tag="mm", name=f"psk{g}")
        for o in range(8):
            nc.tensor.matmul(
                psk[:], c.wk_sb[:, o, :], c.xt_sb[:, o, sl],
                start=(o == 0), stop=(o == 7),
            )
        nc.vector.tensor_copy(c.KTs[b][:, ll], psk[:])
        for h in range(2):
            src = slice(h * 64, (h + 1) * 64)
            nc.vector.tensor_copy(c.KTd[b][h][0:64, ll], c.KTs[b][src, ll])
            nc.vector.tensor_copy(c.KTd[b][h][64:128, ll], c.KTs[b][src, ll])
        yield


def v_units(nc, c, b):
    """V natural [token, vcol] for batch b; per token-tile layout:
    [0:64]=head0 V, 64=ones, [65:129]=head1 V, 129=ones."""
    for tl in range(16):
        t = b * 16 + tl
        psv = c.psA.tile([128, 512], F32, tag="mm", name=f"psv{t}")
        for o in range(8):
            nc.tensor.matmul(
                psv[:, 0:128], c.xt_sb[:, o, t * 128:(t + 1) * 128], c.wv_sb[:, o, :],
                start=(o == 0), stop=(o == 7),
            )
        # one strided copy fills both head halves (cols 0:64 and 65:129)
        nc.vector.tensor_copy(
            c.v_sb[b][:, tl, :].rearrange("p (g x) -> p g x", g=2)[:, :, 0:64],
            psv[:, 0:128].rearrange("p (g x) -> p g x", g=2),
        )
        yield


def epilogue_units(nc, c, b, qg, oT, use_act=False):
    """Normalize + output projection for one 512-query group (4 q-tiles).
    All scale phases (E1) first, then all projection phases (E2), so the
    PE->DVE->PE chains of different q-tiles overlap. use_act routes the PSUM
    evacuations to ScalarE (for the last group, whose epilogue runs in the
    kernel tail where ScalarE is otherwise idle)."""
    copy = nc.scalar.copy if use_act else (lambda o, i: nc.vector.tensor_copy(o, i))
    q0 = qg * 512
    gq0 = b * 2048 + q0
    onats = []
    for j in range(4):
        qj = q0 + j * 128
        ps_st = c.psA.tile([128, 512], F32, tag="mm", name=f"ps_st{b}{qg}{j}")
        nc.tensor.transpose(ps_st[:, 0:64], c.sums_sb[b][:, qj:qj + 128], c.ident64[:])
        recip = c.work.tile([128, 2], F32, tag="recip", name=f"recip{b}{qg}{j}")
        nc.vector.reciprocal(recip[:], ps_st[:, 0:33:32])
        pt1 = c.psA.tile([128, 128], BF16, tag="mm", name=f"pt1_{b}{qg}{j}")
        nc.tensor.transpose(pt1[:], oT[:, j * 128:(j + 1) * 128], c.ident[:])
        onat = c.work.tile([128, 128], BF16, tag=f"onat{j}", name=f"onat{b}{qg}{j}")
        nc.vector.tensor_scalar_mul(onat[:, 0:64], pt1[:, 0:64], recip[:, 0:1])
        nc.vector.tensor_scalar_mul(onat[:, 64:128], pt1[:, 64:128], recip[:, 1:2])
        onats.append(onat)
        yield
    if not use_act:
        for j in range(4):
            gqj = gq0 + j * 128
            pt2 = c.psA.tile([128, 128], BF16, tag="mm", name=f"pt2_{b}{qg}{j}")
            nc.tensor.transpose(pt2[:], onats[j][:], c.ident[:])
            osT = c.work.tile([128, 128], BF16, tag="osT", name=f"osT{b}{qg}{j}")
            copy(osT[:], pt2[:])
            for n in range(2):
                pso = c.psA.tile([128, 512], F32, tag="mm", name=f"pso{b}{qg}{j}{n}")
                nc.tensor.matmul(
                    pso[:], osT[:], c.wo_sb[:, n * 512:(n + 1) * 512],
                    start=True, stop=True,
                )
                ob = c.opool.tile([128, 512], F32, tag="ob", name=f"ob{b}{qg}{j}{n}")
                copy(ob[:], pso[:])
                nc.sync.dma_start(c.out[gqj:gqj + 128, n * 512:(n + 1) * 512], ob[:])
            yield
        return
    # Tail variant: this epilogue drains with nothing left to interleave, so
    # pipeline it explicitly — all transposes+copies first (copies alternating
    # between ScalarE and VectorE), then the projection matmuls with
    # alternating-engine evacuations.
    copies = [nc.scalar.copy, lambda o, i: nc.vector.tensor_copy(o, i)]
    osTs = []
    for j in range(4):
        pt2 = c.psA.tile([128, 128], BF16, tag="mm", name=f"pt2_{b}{qg}{j}")
        nc.tensor.transpose(pt2[:], onats[j][:], c.ident[:])
        osT = c.work.tile([128, 128], BF16, tag=f"osTt{j}", name=f"osT{b}{qg}{j}")
        copies[j % 2](osT[:], pt2[:])
        osTs.append(osT)
        if j % 2:
            yield
    for j in range(4):
        gqj = gq0 + j * 128
        for n in range(2):
            pso = c.psA.tile([128, 512], F32, tag="mm", name=f"pso{b}{qg}{j}{n}")
            nc.tensor.matmul(
                pso[:], osTs[j][:], c.wo_sb[:, n * 512:(n + 1) * 512],
                start=True, stop=True,
            )
            ob = c.opool.tile([128, 512], F32, tag="ob", name=f"ob{b}{qg}{j}{n}")
            copies[n](ob[:], pso[:])
            nc.sync.dma_start(c.out[gqj:gqj + 128, n * 512:(n + 1) * 512], ob[:])
        yield


def emit_s_exp(nc, c, b, qg, h, g2):
    """S^T row-tiled pair + exp for one kt-group; returns the P^T tile."""
    q0 = qg * 512
    ps2 = c.psS.tile([128, 2, 512], F32, tag="s2", name=f"ps2_{b}{qg}{h}{g2}")
    for j in range(2):
        kt = g2 * 2 + j
        rs = slice(j * 64, j * 64 + 64)
        k0 = kt * 128
        nc.tensor.matmul(
            ps2[:, j, :],
            c.KTd[b][h][rs, k0:k0 + 128],
            c.QTd[b][h][rs, q0:q0 + 512],
            start=True, stop=True,
            tile_position=(j * 64, 0),
        )
    pT = c.ptp.tile([128, 2, 512], BF16, tag="pT", name=f"pT{b}{qg}{h}{g2}")
    nc.scalar.activation(pT[:], ps2[:], AF.Exp)
    return pT


def emit_av(nc, c, b, qg, h, po, pT, g2):
    for j in range(2):
        kt = g2 * 2 + j
        nc.tensor.matmul(
            po[0:65, :],
            c.v_sb[b][:, kt, h * 65:h * 65 + 65],
            pT[:, j, :],
            start=(kt == 0), stop=(kt == 15),
        )


def emit_po_evac(nc, c, b, qg, h, po, oT):
    q0 = qg * 512
    nc.vector.tensor_copy(oT[h * 64:(h + 1) * 64, :], po[0:64, :])
    nc.vector.tensor_copy(c.sums_sb[b][h * 32:h * 32 + 1, q0:q0 + 512], po[64:65, :])


def warmup_qg0(nc, c, b, fill):
    """First query group of the first batch: run BOTH heads' S/exp chains while
    the filler emits this batch's QKV projections; all AVs are deferred into a
    filler generator so they interleave with the next query group's backbone."""
    pend = {0: deque(), 1: deque()}
    for g2 in range(8):
        for h in (0, 1):
            pT = emit_s_exp(nc, c, b, 0, h, g2)
            pend[h].append((pT, g2))
            fill.pull(2 if h == 0 else 1)
    oT = c.opool.tile([128, 512], BF16, tag="oT", name=f"oT{b}0")

    def av_burst():
        for h in (0, 1):
            po = c.psO.tile([128, 512], F32, tag="acc", name=f"po{b}0{h}")
            for pT, g2 in pend[h]:
                emit_av(nc, c, b, 0, h, po, pT, g2)
                yield
            emit_po_evac(nc, c, b, 0, h, po, oT)
            yield

    fill.add_front(av_burst())
    fill.add(epilogue_units(nc, c, b, 0, oT))


def stage_b(nc, c, b, fill, warmup_first):
    """Attention backbone for batch b, software-pipelined with filler units."""
    for qg in range(4):
        if warmup_first and qg == 0:
            warmup_qg0(nc, c, b, fill)
            continue
        oT = c.opool.tile([128, 512], BF16, tag="oT", name=f"oT{b}{qg}")
        for h in range(2):
            po = c.psO.tile([128, 512], F32, tag="acc", name=f"po{b}{qg}{h}")
            pend = deque()
            for g2 in range(8):
                pT = emit_s_exp(nc, c, b, qg, h, g2)
                if len(pend) >= 2:
                    emit_av(nc, c, b, qg, h, po, *pend.popleft())
                pend.append((pT, g2))
                fill.pull(1)
            while pend:
                emit_av(nc, c, b, qg, h, po, *pend.popleft())
            emit_po_evac(nc, c, b, qg, h, po, oT)
        fill.add(epilogue_units(nc, c, b, qg, oT, use_act=(b == 1 and qg == 3)))


def build_body(tc, xt, wq, wk, wv, bq, wo, out):
    nc = tc.nc
    c = Ctx()
    c.out = out
    with contextlib.ExitStack() as ctx:
        c.const = ctx.enter_context(tc.tile_pool(name="const", bufs=1))
        c.work = ctx.enter_context(tc.tile_pool(name="work", bufs=3))
        c.ptp = ctx.enter_context(tc.tile_pool(name="ptile", bufs=16))
        c.opool = ctx.enter_context(tc.tile_pool(name="opool", bufs=6))
        # PSUM budget (8 banks): s2 [128,2,512]f32 x2 bufs = 4, mm [128,512]f32
        # x2 bufs = 2, acc x2 = 2.
        c.psA = ctx.enter_context(tc.tile_pool(name="psA", bufs=2, space="PSUM"))
        c.psS = ctx.enter_context(tc.tile_pool(name="psS", bufs=2, space="PSUM"))
        c.psO = ctx.enter_context(tc.tile_pool(name="psO", bufs=2, space="PSUM"))

        # ---- DMA in consumption order: wq/bq, xt quarters 0-1, wk, wv,
        # xt quarters 2-3, wo ----
        c.wq_sb = c.const.tile([128, 8, 128], BF16, name="wq_sb")
        nc.sync.dma_start(c.wq_sb[:], wq[:])
        c.bq_sb = c.const.tile([128, 1], F32, name="bq_sb")
        nc.sync.dma_start(c.bq_sb[:], bq[:])
        actwarm = c.work.tile([1, 1], F32, tag="actwarm", name="actwarm")
        nc.scalar.activation(actwarm[:], c.bq_sb[0:1, 0:1], AF.Exp)

        c.xt_sb = c.const.tile([128, 8, T], BF16, name="xt_sb")
        xtr = xt.rearrange("(o p) t -> p o t", p=128)

        def load_xt(t0, t1):
            for o in range(8):
                nc.sync.dma_start(c.xt_sb[:, o, t0:t1], xtr[:, o, t0:t1])

        load_xt(0, 1024)
        c.wk_sb = c.const.tile([128, 8, 128], BF16, name="wk_sb")
        nc.sync.dma_start(c.wk_sb[:], wk[:])
        c.wv_sb = c.const.tile([128, 8, 128], BF16, name="wv_sb")
        nc.sync.dma_start(c.wv_sb[:], wv[:])
        load_xt(1024, 2048)
        load_xt(2048, 3072)
        load_xt(3072, 4096)
        c.wo_sb = c.const.tile([128, H], BF16, name="wo_sb")
        nc.sync.dma_start(c.wo_sb[:], wo[:])

        c.ident = c.const.tile([128, 128], BF16, name="ident")
        make_identity(nc, c.ident[:])
        c.ident64 = c.const.tile([64, 64], F32, name="ident64")
        make_identity(nc, c.ident64[:])

        # ---- per-batch tensors (disjoint, so batches schedule independently) ----
        c.QTs = [c.const.tile([128, S], BF16, name=f"QTs{b}") for b in range(2)]
        c.KTs = [c.const.tile([128, S], BF16, name=f"KTs{b}") for b in range(2)]
        c.QTd = [[c.const.tile([128, S], BF16, name=f"qtd{b}{h}") for h in range(2)]
                 for b in range(2)]
        c.KTd = [[c.const.tile([128, S], BF16, name=f"ktd{b}{h}") for h in range(2)]
                 for b in range(2)]
        c.v_sb = [c.const.tile([128, 16, 130], BF16, name=f"v_sb{b}") for b in range(2)]
        for b in range(2):
            nc.vector.memset(c.v_sb[b][:, :, 64:130:65], 1.0)
        # softmax sums land on partition 0 (head0) / 32 (head1)
        c.sums_sb = [c.const.tile([64, S], F32, name=f"sums_sb{b}") for b in range(2)]
        for b in range(2):
            nc.vector.memset(c.sums_sb[b][:], 0.0)

        # ---- emission ----
        fill = Filler()
        # First QK group of batch 0 up-front (the backbone needs Q/K group 0).
        qk0 = qk_units(nc, c, 0)
        next(qk0)
        next(qk0)
        fill.add(qk0)              # remaining 6 QK units of batch 0
        fill.add(v_units(nc, c, 0))
        fill.add(qk_units(nc, c, 1))
        fill.add(v_units(nc, c, 1))
        stage_b(nc, c, 0, fill, warmup_first=True)
        stage_b(nc, c, 1, fill, warmup_first=False)
        fill.drain()


def build_program():
    if "nc" in _program_cache:
        return _program_cache["nc"]
    nc = bacc.Bacc("TRN2", target_bir_lowering=False, debug=False)
    xt = nc.dram_tensor("xt", [H, T], BF16, kind="ExternalInput").ap()
    wq = nc.dram_tensor("wq", [128, 8, 128], BF16, kind="ExternalInput").ap()
    wk = nc.dram_tensor("wk", [128, 8, 128], BF16, kind="ExternalInput").ap()
    wv = nc.dram_tensor("wv", [128, 8, 128], BF16, kind="ExternalInput").ap()
    bq = nc.dram_tensor("bq", [128, 1], F32, kind="ExternalInput").ap()
    wo = nc.dram_tensor("wo", [128, H], BF16, kind="ExternalInput").ap()
    out = nc.dram_tensor("out", [T, H], F32, kind="ExternalOutput").ap()
    with tile.TileContext(nc) as tc:
        build_body(tc, xt, wq, wk, wv, bq, wo, out)
    nc.compile()
    _program_cache["nc"] = nc
    return nc


def make_in_maps(x, w_qkv, b_qkv, w_out):
    bf16 = ml_dtypes.bfloat16
    x = np.asarray(x, dtype=np.float32)
    w_qkv = np.asarray(w_qkv, dtype=np.float32)
    b_qkv = np.asarray(b_qkv, dtype=np.float32)
    w_out = np.asarray(w_out, dtype=np.float32)

    xt = np.ascontiguousarray(x.reshape(T, H).T).astype(bf16)  # [H, T]

    def prep_w(w):
        # [1024 hidden, 128] -> SBUF layout [128 part, 8 ktile, 128 col]
        return np.ascontiguousarray(w.reshape(8, 128, 128).transpose(1, 0, 2)).astype(bf16)

    in_maps = []
    for c in range(N_CORES):
        sl = slice(c * 128, (c + 1) * 128)
        in_maps.append({
            "xt": xt,
            "wq": prep_w(w_qkv[:, sl] * 0.125),
            "wk": prep_w(w_qkv[:, H + c * 128:H + (c + 1) * 128]),
            "wv": prep_w(w_qkv[:, 2 * H + c * 128:2 * H + (c + 1) * 128]),
            "bq": (b_qkv[sl] * 0.125).astype(np.float32).reshape(128, 1),
            "wo": np.ascontiguousarray(w_out[sl, :]).astype(bf16),
        })
    return in_maps


def finalize(results, b_qkv, b_out, w_out):
    b_qkv = np.asarray(b_qkv, dtype=np.float32)
    b_out = np.asarray(b_out, dtype=np.float32)
    w_out = np.asarray(w_out, dtype=np.float32)
    acc = np.zeros((T, H), np.float32)
    for r in results:
        acc += np.asarray(r["out"], dtype=np.float32)
    corr = b_out + b_qkv[2 * H:] @ w_out
    return (acc + corr).reshape(B, S, H).astype(np.float32)


def kernel(x, w_qkv, b_qkv, w_out, b_out):
    import os
    # NTFF tracing needs antenv.axon_hooks, which this client env lacks;
    # make sure an inherited BASS_TRACE can't route us into that path.
    os.environ["BASS_NEVER_TRACE"] = "1"
    nc = build_program()
    in_maps = make_in_maps(x, w_qkv, b_qkv, w_out)
    res = run_bass_kernel_spmd(nc, in_maps, list(range(N_CORES)))
    return finalize(res.results, b_qkv, b_out, w_out)



# revision 4
# speedup vs baseline: 1.2153x; 1.2153x over previous
"""Multi-head attention (b=2, s=2048, h=1024, 16 heads x 64) on 8 NeuronCores.

Sharding: tensor-parallel over heads. Core c owns heads {2c, 2c+1} for both
batches. Each core computes a full [4096, 1024] partial of the output
projection (scaled by 1024); the host sums the partials, divides by 1024, and
adds bias corrections (b_out + b_v @ w_out; k-bias dropped: softmax invariant).

All matmuls run in fp8e4m3 DoubleRow perf mode (0.5 cyc/row, 2 k-subtiles per
instruction). Power-of-2 scale management keeps every fp8 tensor in e4m3's
normal range:
  x*8, w_qkv*64, w_out*64 quantized on host.
  Q = psum*2^-5 + 16*bq -> fp8 ; K,V = psum*2^-5 -> fp8  (DVE evac)
  S_psum = 256*(q.k);  P = exp(2^-11 * S_psum) -> fp8    (true logits = qk/8)
  O = sum P*V (psum, col 64 = sum P via ones column in V)
  onat = O * (1/sums) -> bf16  (= 16*attn_out)
  osT  = transpose(onat) -> fp8  (PE transpose via bf16 psum)
  y_psum = osT^T @ (64*w_out) = 1024*y ; evac bf16 ; host /1024.

Dataflow per (batch, qgroup, head) "block" (16 blocks):
  S^T tiles [128 kt, 512 q] via 1 DR matmul each (K=64, zero 2nd subtile),
  exp on ScalarE ([128,2,512] psum -> fp8 P^T in SBUF), some tiles relayed
  DVE-copy -> GPSIMD pow(e^c, S) to offload ScalarE.
  AV in natural orientation: O[128 q, 65] += P^T-slice^T @ V-slice, 8 DR
  chunks, 4 q-subtiles sequentially (single psum bank each).
  Out-projection: 1 DR matmul per [128 q, 512 cols] (both heads contracted).

Engines: ScalarE ~ exp + share of out-evacs; DVE ~ evacs/recip/relay; Pool ~
pow-exp relay + memsets; PE ~50% idle (fp8 made it cheap). The elementwise
engines are the bottleneck, so evac engine assignments are knob-tunable.
"""

import contextlib
import sys
from collections import deque

import numpy as np

sys.path.insert(0, "/opt/trn_rl_repo")

import ml_dtypes  # noqa: E402

import concourse.tile as tile  # noqa: E402
from concourse import bacc, mybir  # noqa: E402
from concourse.bass_utils import run_bass_kernel_spmd  # noqa: E402
from concourse.masks import make_identity  # noqa: E402

BF16 = mybir.dt.bfloat16
F32 = mybir.dt.float32
FP8 = mybir.dt.float8e4
AF = mybir.ActivationFunctionType
DR = mybir.MatmulPerfMode.DoubleRow
f8np = ml_dtypes.float8_e4m3
bfnp = ml_dtypes.bfloat16

B = 2
S = 2048
T = B * S
H = 1024
N_CORES = 8

EXP_SCALE = 2.0 ** -11          # exp(EXP_SCALE * S_psum) = exp(true logits)
QKV_EVAC = 2.0 ** -5            # psum -> fp8 scale for q/k/v
OUT_DIV = 1024.0                # host divides partials by this

# ---- scheduling knobs ----
RELAY_MOD = 5                   # exp idx % RELAY_MOD == RELAY_PHASE -> pool
RELAY_PHASE = 2
OP_EVAC_PAT = "avv"             # engine per OP evac: a=ScalarE v=DVE
QK_EVAC_PAT = "v"
V_EVAC_PAT = "v"
ONAT_PAT = "v"
OST_PAT = "v"

_program_cache = {}


class Ctx:
    pass


class Filler:
    """FIFO of generators; pull() advances the head generator one unit."""

    def __init__(self):
        self.q = deque()

    def add(self, gen):
        self.q.append(gen)

    def add_front(self, gen):
        self.q.appendleft(gen)

    def pull(self, n=1):
        while n > 0 and self.q:
            try:
                next(self.q[0])
                n -= 1
            except StopIteration:
                self.q.popleft()

    def drain(self):
        while self.q:
            self.pull()


def _pick(pat, idx):
    return pat[idx % len(pat)]


def copy_engine(nc, which):
    if which == "a":
        return lambda o, i: nc.scalar.copy(o, i)
    return lambda o, i: nc.vector.tensor_copy(o, i)


# --------------------------------------------------------------------------
# stage A: QKV projections (fillers)
# --------------------------------------------------------------------------

def q_unit(nc, c, b, g):
    """Q^T projection+evac for one 512-token group."""
    sl = slice(g * 512, (g + 1) * 512)
    psq = c.psM.tile([128, 512], F32, tag="mm", name=f"psq{b}{g}")
    for o in range(4):
        nc.tensor.matmul(
            psq[:], c.wq_sb[:, 2 * o:2 * o + 2, :], c.xt_sb[:, 2 * o:2 * o + 2,
            b * 2048 + g * 512:b * 2048 + (g + 1) * 512],
            start=(o == 0), stop=(o == 3), perf_mode=DR,
        )
    nc.vector.tensor_scalar(
        c.qt8[b][:, 0, sl], psq[:], QKV_EVAC, c.bq_sb[:],
        op0=mybir.AluOpType.mult, op1=mybir.AluOpType.add,
    )
    yield


def k_unit(nc, c, b, g):
    sl = slice(g * 512, (g + 1) * 512)
    psk = c.psM.tile([128, 512], F32, tag="mm", name=f"psk{b}{g}")
    for o in range(4):
        nc.tensor.matmul(
            psk[:], c.wk_sb[:, 2 * o:2 * o + 2, :], c.xt_sb[:, 2 * o:2 * o + 2,
            b * 2048 + g * 512:b * 2048 + (g + 1) * 512],
            start=(o == 0), stop=(o == 3), perf_mode=DR,
        )
    nc.vector.tensor_scalar(
        c.kt8[b][:, 0, sl], psk[:], QKV_EVAC, None, op0=mybir.AluOpType.mult,
    )
    yield


def v_units(nc, c, b):
    """V natural [token, vcol] per 128-token tile; v8 layout [128,16,2,65]."""
    for t in range(16):
        psv = c.psM.tile([128, 128], F32, tag="mm", name=f"psv{b}{t}")
        for o in range(4):
            nc.tensor.matmul(
                psv[:], c.xt_sb[:, 2 * o:2 * o + 2,
                b * 2048 + t * 128:b * 2048 + (t + 1) * 128],
                c.wv_sb[:, 2 * o:2 * o + 2, :],
                start=(o == 0), stop=(o == 3), perf_mode=DR,
            )
        nc.vector.tensor_scalar(
            c.v8[b][:, t, :, 0:64],
            psv[:].rearrange("p (h x) -> p h x", h=2),
            QKV_EVAC, None, op0=mybir.AluOpType.mult,
        )
        yield


# --------------------------------------------------------------------------
# attention block: S + exp (backbone), AV + epilogue + OP (fillers)
# --------------------------------------------------------------------------

def emit_s_pair(nc, c, b, qg, h, g2):
    """Two S^T kt tiles [128, 512] into one [128, 2, 512] psum tile."""
    q0 = qg * 512
    hs = slice(h * 64, (h + 1) * 64)
    ps2 = c.psS.tile([128, 2, 512], F32, tag="s2", name=f"ps2_{b}{qg}{h}{g2}")
    for j in range(2):
        kt = g2 * 2 + j
        nc.tensor.matmul(
            ps2[:, j, :],
            c.kt8[b][hs, :, kt * 128:(kt + 1) * 128],
            c.qt8[b][hs, :, q0:q0 + 512],
            start=True, stop=True, perf_mode=DR, tile_position=(h * 64, 0),
        )
    return ps2


def emit_exp(nc, c, ps2, name):
    """exp -> fp8 P^T tile; ScalarE or DVE-copy + Pool pow relay."""
    pT = c.ptp.tile([128, 2, 512], FP8, tag="pT", name=f"pT{name}")
    idx = c.exp_idx
    c.exp_idx += 1
    if idx % RELAY_MOD == RELAY_PHASE:
        scop = c.work.tile([128, 2, 512], BF16, tag="scop", bufs=3,
                           name=f"scop{name}")
        nc.vector.tensor_copy(scop[:], ps2[:])
        nc.gpsimd.tensor_tensor(pT[:], c.econ[:], scop[:], mybir.AluOpType.pow)
    else:
        nc.scalar.activation(pT[:], ps2[:], AF.Exp, scale=EXP_SCALE)
    return pT


def av_epilogue(nc, c, b, qg, h, pts, osts):
    """AV + normalize + transpose for block (b,qg,h); one qsub at a time."""
    for qsub in range(4):
        po = c.psO.tile([128, 512], F32, tag="acc", name=f"po{b}{qg}{h}{qsub}")
        qs = slice(qsub * 128, (qsub + 1) * 128)
        for g2 in range(8):
            nc.tensor.matmul(
                po[:, 0:65],
                pts[g2][:, :, qs],
                c.v8[b][:, 2 * g2:2 * g2 + 2, h, :],
                start=(g2 == 0), stop=(g2 == 7), perf_mode=DR,
            )
        yield
        recip = c.work.tile([128, 1], F32, tag="recip", bufs=4,
                            name=f"rc{b}{qg}{h}{qsub}")
        nc.vector.reciprocal(recip[:], po[:, 64:65])
        onat = c.work.tile([128, 64], BF16, tag="onat", bufs=4,
                           name=f"on{b}{qg}{h}{qsub}")
        nc.vector.tensor_scalar(
            onat[:], po[:, 0:64], recip[:], None, op0=mybir.AluOpType.mult,
        )
        tr = c.psO.tile([128, 128], BF16, tag="acc", name=f"tr{b}{qg}{h}{qsub}")
        nc.tensor.transpose(tr[0:64, :], onat[:], c.ident[:])
        copy_engine(nc, _pick(OST_PAT, c.ost_idx))(
            osts[qsub][0:64, h, :], tr[0:64, :])
        c.ost_idx += 1
        yield


def op_unit(nc, c, b, qg, osts):
    """Out-projection for one (b, qg): 4 qsubs x [128, 1024]."""
    for qsub in range(4):
        gq = b * 2048 + qg * 512 + qsub * 128
        ob = c.work.tile([128, 1024], BF16, tag="ob", bufs=3,
                         name=f"ob{b}{qg}{qsub}")
        for n in range(2):
            psy = c.psM.tile([128, 512], F32, tag="mm", name=f"psy{b}{qg}{qsub}{n}")
            nc.tensor.matmul(
                psy[:], osts[qsub][0:64, :, :],
                c.wo_sb[0:64, :, n * 512:(n + 1) * 512],
                start=True, stop=True, perf_mode=DR,
            )
            copy_engine(nc, _pick(OP_EVAC_PAT, c.op_idx))(
                ob[:, n * 512:(n + 1) * 512], psy[:])
            c.op_idx += 1
        nc.sync.dma_start(c.out[gq:gq + 128, :], ob[:])
        yield


# --------------------------------------------------------------------------

def build_body(tc, xt, wq, wk, wv, bq, wo, out):
    nc = tc.nc
    c = Ctx()
    c.out = out
    c.exp_idx = 0
    c.op_idx = 0
    c.ost_idx = 0
    c.ost_cur = [None] * 4
    c.ost_prev = [None] * 4
    with contextlib.ExitStack() as ctx:
        c.const = ctx.enter_context(tc.tile_pool(name="const", bufs=1))
        c.work = ctx.enter_context(tc.tile_pool(name="work", bufs=3))
        c.ptp = ctx.enter_context(tc.tile_pool(name="ptile", bufs=16))
        # PSUM (8 banks): s2 [128,2,512]f32 x2 = 4, acc(+tr) x2 = 2, mm x2 = 2
        c.psS = ctx.enter_context(tc.tile_pool(name="psS", bufs=2, space="PSUM"))
        c.psO = ctx.enter_context(tc.tile_pool(name="psO", bufs=2, space="PSUM"))
        c.psM = ctx.enter_context(tc.tile_pool(name="psM", bufs=2, space="PSUM"))

        # ---- DMA in consumption order ----
        c.wq_sb = c.const.tile([128, 8, 128], FP8, name="wq_sb")
        nc.sync.dma_start(c.wq_sb[:], wq[:])
        c.bq_sb = c.const.tile([128, 1], F32, name="bq_sb")
        nc.sync.dma_start(c.bq_sb[:], bq[:])
        actwarm = c.work.tile([1, 1], F32, tag="actwarm", bufs=1, name="actwarm")
        nc.scalar.activation(actwarm[:], c.bq_sb[0:1, 0:1], AF.Exp)

        c.xt_sb = c.const.tile([128, 8, T], FP8, name="xt_sb")

        def load_xt(t0, t1):
            nc.sync.dma_start(c.xt_sb[:, :, t0:t1], xt[:, :, t0:t1])

        load_xt(0, 512)
        c.wk_sb = c.const.tile([128, 8, 128], FP8, name="wk_sb")
        nc.sync.dma_start(c.wk_sb[:], wk[:])
        c.wv_sb = c.const.tile([128, 8, 128], FP8, name="wv_sb")
        nc.sync.dma_start(c.wv_sb[:], wv[:])
        load_xt(512, 1024)
        load_xt(1024, 2048)
        c.wo_sb = c.const.tile([64, 2, 1024], FP8, name="wo_sb")
        nc.sync.dma_start(c.wo_sb[:], wo[:])
        load_xt(2048, 3072)
        load_xt(3072, 4096)

        c.ident = c.const.tile([128, 128], BF16, name="ident")
        make_identity(nc, c.ident[:])
        c.econ = c.const.tile([128, 2, 512], F32, name="econ")
        nc.gpsimd.memset(c.econ[:], float(np.exp(EXP_SCALE)))

        # per-batch fp8 operand tensors
        c.qt8 = [c.const.tile([128, 2, S], FP8, name=f"qt8_{b}") for b in range(2)]
        c.kt8 = [c.const.tile([128, 2, S], FP8, name=f"kt8_{b}") for b in range(2)]
        c.v8 = [c.const.tile([128, 16, 2, 65], FP8, name=f"v8_{b}") for b in range(2)]
        for b in range(2):
            nc.gpsimd.memset(c.qt8[b][:, 1, :], 0.0)   # zero DR subtile
            nc.gpsimd.memset(c.kt8[b][:, 1, :], 0.0)
            nc.vector.memset(c.v8[b][:, :, :, 64:65], 1.0)  # softmax-sum ones

        # ---- emission ----
        fill = Filler()
        # prefill: K0, Q0 of batch 0 emitted up-front (S backbone needs them)
        for _ in k_unit(nc, c, 0, 0):
            pass
        for _ in q_unit(nc, c, 0, 0):
            pass
        for g in range(1, 4):
            fill.add(k_unit(nc, c, 0, g))
        fill.add(v_units(nc, c, 0))
        for g in range(1, 4):
            fill.add(q_unit(nc, c, 0, g))
        for g in range(4):
            fill.add(k_unit(nc, c, 1, g))
        fill.add(q_unit(nc, c, 1, 0))
        fill.add(v_units(nc, c, 1))
        for g in range(1, 4):
            fill.add(q_unit(nc, c, 1, g))

        blocks = [(b, qg, h) for b in range(2) for qg in range(4) for h in range(2)]
        osts_map = {}

        def osts_for(b, qg):
            if (b, qg) not in osts_map:
                osts_map[(b, qg)] = [
                    c.work.tile([64, 2, 128], FP8, tag=f"osT{q}", bufs=2,
                                name=f"osT{b}{qg}{q}")
                    for q in range(4)
                ]
            return osts_map[(b, qg)]

        pts_prev = None
        prev_blk = None
        for n, (b, qg, h) in enumerate(blocks):
            pts = []
            if prev_blk is not None:
                fill.add_front(av_epilogue(nc, c, *prev_blk, pts_prev,
                                           osts_for(prev_blk[0], prev_blk[1])))
            for g2 in range(8):
                ps2 = emit_s_pair(nc, c, b, qg, h, g2)
                pts.append(emit_exp(nc, c, ps2, f"{b}{qg}{h}{g2}"))
                fill.pull(2)
            if prev_blk is not None and prev_blk[2] == 1:
                # both heads of (prev b, prev qg) done once its epilogue runs
                fill.add(op_unit(nc, c, prev_blk[0], prev_blk[1],
                                 osts_for(prev_blk[0], prev_blk[1])))
            pts_prev = pts
            prev_blk = (b, qg, h)
        fill.add_front(av_epilogue(nc, c, *prev_blk, pts_prev,
                                   osts_for(prev_blk[0], prev_blk[1])))
        fill.add(op_unit(nc, c, prev_blk[0], prev_blk[1],
                         osts_for(prev_blk[0], prev_blk[1])))
        fill.drain()


def build_program():
    key = (RELAY_MOD, RELAY_PHASE, OP_EVAC_PAT, QK_EVAC_PAT, V_EVAC_PAT,
           ONAT_PAT, OST_PAT)
    if key in _program_cache:
        return _program_cache[key]
    nc = bacc.Bacc("TRN2", target_bir_lowering=False, debug=False)
    xt = nc.dram_tensor("xt", [128, 8, T], FP8, kind="ExternalInput").ap()
    wq = nc.dram_tensor("wq", [128, 8, 128], FP8, kind="ExternalInput").ap()
    wk = nc.dram_tensor("wk", [128, 8, 128], FP8, kind="ExternalInput").ap()
    wv = nc.dram_tensor("wv", [128, 8, 128], FP8, kind="ExternalInput").ap()
    bq = nc.dram_tensor("bq", [128, 1], F32, kind="ExternalInput").ap()
    wo = nc.dram_tensor("wo", [64, 2, 1024], FP8, kind="ExternalInput").ap()
    out = nc.dram_tensor("out", [T, H], BF16, kind="ExternalOutput").ap()
    with tile.TileContext(nc) as tc:
        build_body(tc, xt, wq, wk, wv, bq, wo, out)
    nc.compile()
    _program_cache[key] = nc
    return nc


def make_in_maps(x, w_qkv, b_qkv, w_out):
    x = np.asarray(x, dtype=np.float32)
    w_qkv = np.asarray(w_qkv, dtype=np.float32)
    b_qkv = np.asarray(b_qkv, dtype=np.float32)
    w_out = np.asarray(w_out, dtype=np.float32)

    # x^T [H, T] scaled by 8, in [128, 8, T] layout (hidden ktile on dim1)
    xt = np.ascontiguousarray(
        (x.reshape(T, H).T * 8.0).reshape(8, 128, T).transpose(1, 0, 2)
    ).astype(f8np)

    def prep_w(w):
        # [1024 hidden, 128 cols] -> [128 part, 8 ktile, 128 col], *64
        return np.ascontiguousarray(
            (w * 64.0).reshape(8, 128, 128).transpose(1, 0, 2)
        ).astype(f8np)

    in_maps = []
    for cc in range(N_CORES):
        sl = slice(cc * 128, (cc + 1) * 128)
        wo_c = np.ascontiguousarray(
            (w_out[sl, :] * 64.0).reshape(2, 64, H).transpose(1, 0, 2)
        ).astype(f8np)
        in_maps.append({
            "xt": xt,
            "wq": prep_w(w_qkv[:, sl]),
            "wk": prep_w(w_qkv[:, H + cc * 128:H + (cc + 1) * 128]),
            "wv": prep_w(w_qkv[:, 2 * H + cc * 128:2 * H + (cc + 1) * 128]),
            "bq": (b_qkv[sl] * 16.0).astype(np.float32).reshape(128, 1),
            "wo": wo_c,
        })
    return in_maps


def finalize(results, b_qkv, b_out, w_out):
    b_qkv = np.asarray(b_qkv, dtype=np.float32)
    b_out = np.asarray(b_out, dtype=np.float32)
    w_out = np.asarray(w_out, dtype=np.float32)
    acc = np.zeros((T, H), np.float32)
    for r in results:
        acc += np.asarray(r["out"], dtype=np.float32)
    acc /= OUT_DIV
    corr = b_out + b_qkv[2 * H:] @ w_out
    return (acc + corr).reshape(B, S, H).astype(np.float32)


def kernel(x, w_qkv, b_qkv, w_out, b_out):
    import os

    os.environ["BASS_NEVER_TRACE"] = "1"
    nc = build_program()
    in_maps = make_in_maps(x, w_qkv, b_qkv, w_out)
    res = run_bass_kernel_spmd(nc, in_maps, list(range(N_CORES)))
    return finalize(res.results, b_qkv, b_out, w_out)


# revision 20
# speedup vs baseline: 1.3723x; 1.1292x over previous
"""Multi-head attention (b=2, s=2048, h=1024, 16 heads x 64) on 8 NeuronCores.

Sharding: tensor-parallel over heads. Core c owns heads {2c, 2c+1} for both
batches. Each core computes a full [4096, 1024] partial of the output
projection (scaled by 1024); the host sums the partials, divides by 1024, and
adds bias corrections (b_out + b_v @ w_out; k-bias dropped: softmax invariant).

All matmuls run in fp8e4m3 DoubleRow perf mode (0.5 cyc/row, 2 k-subtiles per
instruction). Power-of-2 scale management keeps every fp8 tensor in e4m3's
normal range:
  x*8, w_qkv*64, w_out*64 quantized on host.
  Q = psum*2^-5 + 16*bq -> fp8 ; K,V = psum*2^-5 -> fp8  (DVE evac)
  S_psum = 256*(q.k);  P = exp(2^-11 * S_psum) -> fp8    (true logits = qk/8)
  O = sum P*V (psum, col 64 = sum P via ones column in V)
  onat = O * (1/sums) -> bf16  (= 16*attn_out)
  osT  = transpose(onat) -> fp8  (PE transpose via bf16 psum)
  y_psum = osT^T @ (64*w_out) = 1024*y ; evac bf16 ; host /1024.

Dataflow per (batch, qgroup, head) "block" (16 blocks):
  S^T tiles [128 kt, 512 q] via 1 DR matmul each (K=64, zero 2nd subtile),
  exp on ScalarE ([128,2,512] psum -> fp8 P^T in SBUF), some tiles relayed
  DVE-copy -> GPSIMD pow(e^c, S) to offload ScalarE.
  AV in natural orientation: O[128 q, 65] += P^T-slice^T @ V-slice, 8 DR
  chunks, 4 q-subtiles sequentially (single psum bank each).
  Out-projection: 1 DR matmul per [128 q, 512 cols] (both heads contracted).

Engines: ScalarE ~ exp + share of out-evacs; DVE ~ evacs/recip/relay; Pool ~
pow-exp relay + memsets; PE ~50% idle (fp8 made it cheap). The elementwise
engines are the bottleneck, so evac engine assignments are knob-tunable.
"""

import contextlib
import sys
from collections import deque

import numpy as np

sys.path.insert(0, "/opt/trn_rl_repo")

import ml_dtypes  # noqa: E402

import concourse.tile as tile  # noqa: E402
from concourse import bacc, mybir  # noqa: E402
from concourse.bass_utils import run_bass_kernel_spmd  # noqa: E402
from concourse.masks import make_identity  # noqa: E402

BF16 = mybir.dt.bfloat16
F32 = mybir.dt.float32
FP8 = mybir.dt.float8e4
AF = mybir.ActivationFunctionType
DR = mybir.MatmulPerfMode.DoubleRow
f8np = ml_dtypes.float8_e4m3
bfnp = ml_dtypes.bfloat16

B = 2
S = 2048
T = B * S
H = 1024
N_CORES = 8

EXP_SCALE = 2.0 ** -11          # exp(EXP_SCALE * S_psum) = exp(true logits)
QKV_EVAC = 2.0 ** -5            # psum -> fp8 scale for q/k/v
OUT_DIV = 1024.0                # host divides partials by this

# ---- scheduling knobs ----
RELAY_MOD = 8                   # exp idx % RELAY_MOD == RELAY_PHASE -> pool
RELAY_PHASE = 4
OP_EVAC_PAT = "v"               # engine per OP evac: a=ScalarE v=DVE
QK_EVAC_PAT = "v"
V_EVAC_PAT = "v"
ONAT_PAT = "v"
OST_PAT = "v"

_program_cache = {}


class Ctx:
    pass


class Gen:
    """Generator wrapper with exhaustion flag."""

    def __init__(self, gen):
        self.gen = gen
        self.done = False

    def step(self):
        if self.done:
            return False
        try:
            next(self.gen)
            return True
        except StopIteration:
            self.done = True
            return False


class Filler:
    """FIFO of Gen wrappers; pull() advances the head generator one unit."""

    def __init__(self):
        self.q = deque()

    def add(self, gen):
        g = Gen(gen) if not isinstance(gen, Gen) else gen
        self.q.append(g)
        return g

    def add_front(self, gen):
        g = Gen(gen) if not isinstance(gen, Gen) else gen
        self.q.appendleft(g)
        return g

    def pull(self, n=1):
        while n > 0 and self.q:
            if self.q[0].step():
                n -= 1
            else:
                self.q.popleft()

    def drain_until(self, g):
        """Drain from the head until generator g is exhausted (g must be in
        the queue; everything ahead of it drains fully)."""
        while not g.done:
            if not self.q:
                raise RuntimeError("drain_until: generator not in queue")
            self.pull(1)

    def drain(self):
        while self.q:
            self.pull()


def _pick(pat, idx):
    return pat[idx % len(pat)]


def copy_engine(nc, which):
    if which == "a":
        return lambda o, i: nc.scalar.copy(o, i)
    return lambda o, i: nc.vector.tensor_copy(o, i)


# --------------------------------------------------------------------------
# stage A: QKV projections (fillers)
# --------------------------------------------------------------------------

def q_unit(nc, c, b, t0=None, t1=None, g=None):
    """Q^T projection+evac for a token range (default one 512-token group)."""
    if g is not None:
        t0, t1 = g * 512, (g + 1) * 512
    psq = c.psM.tile([128, t1 - t0], F32, tag="mm", name=f"psq{b}{t0}")
    for o in range(4):
        nc.tensor.matmul(
            psq[:], c.wq_sb[:, 2 * o:2 * o + 2, :],
            c.xt_sb[:, 2 * o:2 * o + 2, b * 2048 + t0:b * 2048 + t1],
            start=(o == 0), stop=(o == 3), perf_mode=DR,
        )
    nc.vector.tensor_scalar(
        c.qt8[b][:, 0, t0:t1], psq[:], QKV_EVAC, c.bq_sb[:],
        op0=mybir.AluOpType.mult, op1=mybir.AluOpType.add,
    )
    yield


def k_unit(nc, c, b, t0=None, t1=None, g=None):
    if g is not None:
        t0, t1 = g * 512, (g + 1) * 512
    psk = c.psM.tile([128, t1 - t0], F32, tag="mm", name=f"psk{b}{t0}")
    for o in range(4):
        nc.tensor.matmul(
            psk[:], c.wk_sb[:, 2 * o:2 * o + 2, :],
            c.xt_sb[:, 2 * o:2 * o + 2, b * 2048 + t0:b * 2048 + t1],
            start=(o == 0), stop=(o == 3), perf_mode=DR,
        )
    nc.vector.tensor_scalar(
        c.kt8[b][:, 0, t0:t1], psk[:], QKV_EVAC, None, op0=mybir.AluOpType.mult,
    )
    yield


def v_units(nc, c, b):
    """V natural [token, vcol] per 128-token tile; v8 layout [128,16,2,65]."""
    for t in range(16):
        psv = c.psM.tile([128, 128], F32, tag="mm", name=f"psv{b}{t}")
        for o in range(4):
            nc.tensor.matmul(
                psv[:], c.xt_sb[:, 2 * o:2 * o + 2,
                b * 2048 + t * 128:b * 2048 + (t + 1) * 128],
                c.wv_sb[:, 2 * o:2 * o + 2, :],
                start=(o == 0), stop=(o == 3), perf_mode=DR,
            )
        nc.vector.tensor_scalar(
            c.v8[b][:, t, :, 0:64],
            psv[:].rearrange("p (h x) -> p h x", h=2),
            QKV_EVAC, None, op0=mybir.AluOpType.mult,
        )
        yield


# --------------------------------------------------------------------------
# attention block: S + exp (backbone), AV + epilogue + OP (fillers)
# --------------------------------------------------------------------------

def emit_s_exp(nc, c, b, qg, h, g2):
    """Two S^T kt tiles + exp -> fp8 P^T tile [128, 2, 512].

    ScalarE path: S pair into one psS [128,2,512] tile, one Exp activation.
    Relay path (every RELAY_MOD-th): S pair into two psM [128,512] tiles
    (keeps the psS ring private to the ScalarE exp chain), DVE-copy to bf16,
    pow(e^c, S) on GPSIMD.
    """
    q0 = qg * 512
    hs = slice(h * 64, (h + 1) * 64)
    name = f"{b}{qg}{h}{g2}"
    idx = c.exp_idx
    c.exp_idx += 1
    pT = c.ptp.tile([128, 2, 512], FP8, tag="pT", name=f"pT{name}")
    relay = idx % RELAY_MOD == RELAY_PHASE

    def s_mm(out, j):
        kt = g2 * 2 + j
        nc.tensor.matmul(
            out,
            c.kt8[b][hs, :, kt * 128:(kt + 1) * 128],
            c.qt8[b][hs, :, q0:q0 + 512],
            start=True, stop=True, perf_mode=DR, tile_position=(h * 64, 0),
        )

    if relay:
        scop = c.work.tile([128, 2, 512], F32, tag="scop", bufs=3,
                           name=f"scop{name}")
        for j in range(2):
            psj = c.psM.tile([128, 512], F32, tag="mm", name=f"psr{name}{j}")
            s_mm(psj[:], j)
            nc.vector.tensor_copy(scop[:, j, :], psj[:])
        nc.gpsimd.tensor_tensor(pT[:], c.econ[:], scop[:], mybir.AluOpType.pow)
    else:
        ps2 = c.psS.tile([128, 2, 512], F32, tag="s2", name=f"ps2_{name}")
        for j in range(2):
            s_mm(ps2[:, j, :], j)
        nc.scalar.activation(pT[:], ps2[:], AF.Exp, scale=EXP_SCALE)
    return pT


def av_epilogue(nc, c, b, qg, h, pts, osts, tail=False):
    """AV + normalize + transpose for block (b,qg,h); one qsub at a time.

    tail=True: spread the copies/scales across ScalarE too (kernel drain,
    ScalarE otherwise idle)."""
    for qsub in range(4):
        po = c.psO.tile([128, 512], F32, tag="acc", name=f"po{b}{qg}{h}{qsub}")
        qs = slice(qsub * 128, (qsub + 1) * 128)
        for g2 in range(8):
            nc.tensor.matmul(
                po[:, 0:65],
                pts[g2][:, :, qs],
                c.v8[b][:, 2 * g2:2 * g2 + 2, h, :],
                start=(g2 == 0), stop=(g2 == 7), perf_mode=DR,
            )
        yield
        recip = c.work.tile([128, 1], F32, tag="recip", bufs=4,
                            name=f"rc{b}{qg}{h}{qsub}")
        nc.vector.reciprocal(recip[:], po[:, 64:65])
        onat = c.work.tile([128, 64], BF16, tag="onat", bufs=4,
                           name=f"on{b}{qg}{h}{qsub}")
        if tail and qsub % 2 == 0:
            nc.scalar.activation(onat[:], po[:, 0:64], AF.Identity,
                                 scale=recip[:])
        else:
            nc.vector.tensor_scalar(
                onat[:], po[:, 0:64], recip[:], None, op0=mybir.AluOpType.mult,
            )
        tr = c.psO.tile([128, 128], BF16, tag="acc", name=f"tr{b}{qg}{h}{qsub}")
        nc.tensor.transpose(tr[0:64, :], onat[:], c.ident[:])
        ost_eng = ("a" if qsub % 2 else "v") if tail else _pick(OST_PAT, c.ost_idx)
        copy_engine(nc, ost_eng)(osts[qsub][0:64, h, :], tr[0:64, :])
        c.ost_idx += 1
        yield


def op_unit(nc, c, b, qg, osts, tail=False):
    """Out-projection for one (b, qg): 4 qsubs x [128, 1024]."""
    for qsub in range(4):
        gq = b * 2048 + qg * 512 + qsub * 128
        ob = c.work.tile([128, 1024], BF16, tag="ob", bufs=3,
                         name=f"ob{b}{qg}{qsub}")
        for n in range(2):
            psy = c.psM.tile([128, 512], F32, tag="mm", name=f"psy{b}{qg}{qsub}{n}")
            nc.tensor.matmul(
                psy[:], osts[qsub][0:64, :, :],
                c.wo_sb[0:64, :, n * 512:(n + 1) * 512],
                start=True, stop=True, perf_mode=DR,
            )
            eng = ("a" if n else "v") if tail else _pick(OP_EVAC_PAT, c.op_idx)
            copy_engine(nc, eng)(ob[:, n * 512:(n + 1) * 512], psy[:])
            c.op_idx += 1
        nc.sync.dma_start(c.out[gq:gq + 128, :], ob[:])
        yield


# --------------------------------------------------------------------------

def build_body(tc, xt, wq, wk, wv, bq, wo, out):
    nc = tc.nc
    c = Ctx()
    c.out = out
    c.exp_idx = 0
    c.op_idx = 0
    c.ost_idx = 0
    c.ost_cur = [None] * 4
    c.ost_prev = [None] * 4
    with contextlib.ExitStack() as ctx:
        c.const = ctx.enter_context(tc.tile_pool(name="const", bufs=1))
        c.work = ctx.enter_context(tc.tile_pool(name="work", bufs=3))
        c.ptp = ctx.enter_context(tc.tile_pool(name="ptile", bufs=16))
        # PSUM (8 banks): s2 [128,2,512]f32 x2 = 4, acc(+tr) x2 = 2, mm x2 = 2
        c.psS = ctx.enter_context(tc.tile_pool(name="psS", bufs=2, space="PSUM"))
        c.psO = ctx.enter_context(tc.tile_pool(name="psO", bufs=2, space="PSUM"))
        c.psM = ctx.enter_context(tc.tile_pool(name="psM", bufs=2, space="PSUM"))

        # ---- DMA in consumption order ----
        c.wq_sb = c.const.tile([128, 8, 128], FP8, name="wq_sb")
        nc.sync.dma_start(c.wq_sb[:], wq[:])
        c.bq_sb = c.const.tile([128, 1], F32, name="bq_sb")
        nc.sync.dma_start(c.bq_sb[:], bq[:])
        actwarm = c.work.tile([1, 1], F32, tag="actwarm", bufs=1, name="actwarm")
        nc.scalar.activation(actwarm[:], c.bq_sb[0:1, 0:1], AF.Exp)

        c.xt_sb = c.const.tile([128, 8, T], FP8, name="xt_sb")

        def load_xt(t0, t1):
            nc.sync.dma_start(c.xt_sb[:, :, t0:t1], xt[:, :, t0:t1])

        c.wk_sb = c.const.tile([128, 8, 128], FP8, name="wk_sb")
        nc.sync.dma_start(c.wk_sb[:], wk[:])
        load_xt(0, 256)
        load_xt(256, 512)
        c.wv_sb = c.const.tile([128, 8, 128], FP8, name="wv_sb")
        nc.sync.dma_start(c.wv_sb[:], wv[:])
        load_xt(512, 1024)
        load_xt(1024, 2048)
        c.wo_sb = c.const.tile([64, 2, 1024], FP8, name="wo_sb")
        nc.sync.dma_start(c.wo_sb[:], wo[:])
        load_xt(2048, 3072)
        load_xt(3072, 4096)

        c.ident = c.const.tile([128, 128], BF16, name="ident")
        make_identity(nc, c.ident[:])
        c.econ = c.const.tile([128, 2, 512], F32, name="econ")
        nc.gpsimd.memset(c.econ[:], float(np.exp(EXP_SCALE)))

        # per-batch fp8 operand tensors
        c.qt8 = [c.const.tile([128, 2, S], FP8, name=f"qt8_{b}") for b in range(2)]
        c.kt8 = [c.const.tile([128, 2, S], FP8, name=f"kt8_{b}") for b in range(2)]
        c.v8 = [c.const.tile([128, 16, 2, 65], FP8, name=f"v8_{b}") for b in range(2)]
        for b in range(2):
            nc.gpsimd.memset(c.qt8[b][:, 1, :], 0.0)   # zero DR subtile
            nc.gpsimd.memset(c.kt8[b][:, 1, :], 0.0)
            nc.vector.memset(c.v8[b][:, :, :, 64:65], 1.0)  # softmax-sum ones

        # ---- emission ----
        fill = Filler()
        # prefill: K0, Q0 of batch 0 up-front, fine-grained to chase the
        # first xt DMA chunks (K0a only needs tokens 0:256)
        for rng in ((k_unit, 0, 256), (q_unit, 0, 512), (k_unit, 256, 512)):
            for _ in rng[0](nc, c, 0, rng[1], rng[2]):
                pass
        k_gens = {0: [], 1: []}
        q_gens = {}
        v_gens = {}
        for g in range(1, 4):
            k_gens[0].append(fill.add(k_unit(nc, c, 0, g=g)))
        v_gens[0] = fill.add(v_units(nc, c, 0))
        for g in range(1, 4):
            q_gens[(0, g)] = fill.add(q_unit(nc, c, 0, g=g))
        for g in range(4):
            k_gens[1].append(fill.add(k_unit(nc, c, 1, g=g)))
        q_gens[(1, 0)] = fill.add(q_unit(nc, c, 1, g=0))
        v_gens[1] = fill.add(v_units(nc, c, 1))
        for g in range(1, 4):
            q_gens[(1, g)] = fill.add(q_unit(nc, c, 1, g=g))

        blocks = [(b, qg, h) for b in range(2) for qg in range(4) for h in range(2)]
        osts_map = {}

        def osts_for(b, qg):
            if (b, qg) not in osts_map:
                osts_map[(b, qg)] = [
                    c.work.tile([64, 2, 128], FP8, tag=f"osT{q}", bufs=2,
                                name=f"osT{b}{qg}{q}")
                    for q in range(4)
                ]
            return osts_map[(b, qg)]

        pts_prev = None
        prev_blk = None
        for n, (b, qg, h) in enumerate(blocks):
            # emission-order prerequisites: the epilogue of prev_blk reads
            # v8[prev_b]; this block's S matmuls read kt8/qt8 slices.
            if prev_blk is not None:
                fill.drain_until(v_gens[prev_blk[0]])
            for kg in k_gens[b]:
                fill.drain_until(kg)
            if (b, qg) in q_gens:
                fill.drain_until(q_gens[(b, qg)])
            pts = []
            if prev_blk is not None:
                fill.add_front(av_epilogue(nc, c, *prev_blk, pts_prev,
                                           osts_for(prev_blk[0], prev_blk[1])))
            for g2 in range(8):
                pts.append(emit_s_exp(nc, c, b, qg, h, g2))
                fill.pull(2)
            if prev_blk is not None and prev_blk[2] == 1:
                # both heads of (prev b, prev qg) done once its epilogue runs
                fill.add(op_unit(nc, c, prev_blk[0], prev_blk[1],
                                 osts_for(prev_blk[0], prev_blk[1])))
            pts_prev = pts
            prev_blk = (b, qg, h)
        fill.add_front(av_epilogue(nc, c, *prev_blk, pts_prev,
                                   osts_for(prev_blk[0], prev_blk[1]),
                                   tail=True))
        fill.add(op_unit(nc, c, prev_blk[0], prev_blk[1],
                         osts_for(prev_blk[0], prev_blk[1]), tail=True))
        fill.drain()


def build_program():
    key = (RELAY_MOD, RELAY_PHASE, OP_EVAC_PAT, QK_EVAC_PAT, V_EVAC_PAT,
           ONAT_PAT, OST_PAT)
    if key in _program_cache:
        return _program_cache[key]
    nc = bacc.Bacc("TRN2", target_bir_lowering=False, debug=False)
    xt = nc.dram_tensor("xt", [128, 8, T], FP8, kind="ExternalInput").ap()
    wq = nc.dram_tensor("wq", [128, 8, 128], FP8, kind="ExternalInput").ap()
    wk = nc.dram_tensor("wk", [128, 8, 128], FP8, kind="ExternalInput").ap()
    wv = nc.dram_tensor("wv", [128, 8, 128], FP8, kind="ExternalInput").ap()
    bq = nc.dram_tensor("bq", [128, 1], F32, kind="ExternalInput").ap()
    wo = nc.dram_tensor("wo", [64, 2, 1024], FP8, kind="ExternalInput").ap()
    out = nc.dram_tensor("out", [T, H], BF16, kind="ExternalOutput").ap()
    with tile.TileContext(nc) as tc:
        build_body(tc, xt, wq, wk, wv, bq, wo, out)
    nc.compile()
    _program_cache[key] = nc
    return nc


def make_in_maps(x, w_qkv, b_qkv, w_out):
    x = np.asarray(x, dtype=np.float32)
    w_qkv = np.asarray(w_qkv, dtype=np.float32)
    b_qkv = np.asarray(b_qkv, dtype=np.float32)
    w_out = np.asarray(w_out, dtype=np.float32)

    # x^T [H, T] scaled by 8, in [128, 8, T] layout (hidden ktile on dim1)
    xt = np.ascontiguousarray(
        (x.reshape(T, H).T * 8.0).reshape(8, 128, T).transpose(1, 0, 2)
    ).astype(f8np)

    def prep_w(w):
        # [1024 hidden, 128 cols] -> [128 part, 8 ktile, 128 col], *64
        return np.ascontiguousarray(
            (w * 64.0).reshape(8, 128, 128).transpose(1, 0, 2)
        ).astype(f8np)

    in_maps = []
    for cc in range(N_CORES):
        sl = slice(cc * 128, (cc + 1) * 128)
        wo_c = np.ascontiguousarray(
            (w_out[sl, :] * 64.0).reshape(2, 64, H).transpose(1, 0, 2)
        ).astype(f8np)
        in_maps.append({
            "xt": xt,
            "wq": prep_w(w_qkv[:, sl]),
            "wk": prep_w(w_qkv[:, H + cc * 128:H + (cc + 1) * 128]),
            "wv": prep_w(w_qkv[:, 2 * H + cc * 128:2 * H + (cc + 1) * 128]),
            "bq": (b_qkv[sl] * 16.0).astype(np.float32).reshape(128, 1),
            "wo": wo_c,
        })
    return in_maps


def finalize(results, b_qkv, b_out, w_out):
    b_qkv = np.asarray(b_qkv, dtype=np.float32)
    b_out = np.asarray(b_out, dtype=np.float32)
    w_out = np.asarray(w_out, dtype=np.float32)
    acc = np.zeros((T, H), np.float32)
    for r in results:
        acc += np.asarray(r["out"], dtype=np.float32)
    acc /= OUT_DIV
    corr = b_out + b_qkv[2 * H:] @ w_out
    return (acc + corr).reshape(B, S, H).astype(np.float32)


def kernel(x, w_qkv, b_qkv, w_out, b_out):
    import os

    os.environ["BASS_NEVER_TRACE"] = "1"
    nc = build_program()
    in_maps = make_in_maps(x, w_qkv, b_qkv, w_out)
    res = run_bass_kernel_spmd(nc, in_maps, list(range(N_CORES)))
    return finalize(res.results, b_qkv, b_out, w_out)
